# revision 1
# baseline (speedup 1.0000x reference)
"""Multi-Head Latent Attention (MLA) prefill kernel for 8 Trainium2 NeuronCores.

Problem shapes: B=2, S=2048, DIM=2048, H=16 heads, HEAD_DIM=128, LATENT=512.

Strategy (tensor-parallel over heads + data-parallel phase A):
  Phase A (token-DP): each core computes the latent down-projections
    c_kvT (fp16), c_qT (fp8 via DoubleRow over embedding-chunk pairs)
    and the rope projections k_rT/q_rT (pe-scaled, emitted fp8) for its
    512-token shard in transposed [feature, token] layout. ec-outer
    loop order streams the matmuls behind the input DMAs, which are
    split across the SP (x) and ACT (weights) hardware DGE queues.
  AllGather x2 (ckv fp16; cq + ropes fp8, riding the fp16 bounce
    buffer as raw bytes via AP bitcast).
  Phase B (head-TP, 2 heads/core):
    kc/qc up-projected into fp8 "pair" tiles (3 slots: kc_h0, kc_h1,
    kr shared -> strided slot pairs feed DoubleRow directly); v stays
    fp16 with a ones-column so the softmax denominator falls out of
    the ctx matmul for free.
    Scores: ONE DoubleRow fp8 matmul per 128-key chunk fuses the
    content (kc.qc) and rope (kr.qr) contractions (virtual K=256).
    The attention is software-pipelined over 16 (batch, q-block, head)
    blocks: scores/exp of block i issue before ctx chains of block
    i-1, transposes of block i-2, and one deferred out-projection, so
    the PE never waits on the ACT exp stream. Only batch-0 kc/v and
    the first q-block are up-projected before attention starts; the
    rest ride the prologue's PE slack (late kc in fp16 to skip the
    fp8 convert; late bias-adds on DVE to keep ACT free).
  Host: sums the 8 partial outputs and adds b_O.

fp8 scope was chosen empirically (numpy e4m3 emulation matches HW to
<1e-3 every time): scores+up-proj+c_q fp8 and fp8 est (exp scores,
which also makes the 1024 ctx-matmul LDWEIGHTS stream at 4B/cycle)
-> rel_err 1.52e-2 vs the 2e-2 budget. The V-path and the output
projection stay fp16: each would push past the budget. ~246 us/core
device time, 243.6 us after the leading-chunk DMA split
(TimelineSim; reps-differential HW measurement agrees
within noise) vs 618-691 us for the fp16 baseline.
"""
import math

import numpy as np

import concourse.bacc as bacc
import concourse.mybir as mybir
import concourse.tile as tile
from concourse.masks import make_identity

# Problem constants (hardcoded per harness contract).
B, S, DIM, H, HD, LAT = 2, 2048, 2048, 16, 128, 512
N_CORES = 8
HPC = H // N_CORES          # heads per core = 2
TOK = B * S                 # 4096 tokens
TPC = TOK // N_CORES        # 512 tokens per core (phase A shard)
EC = DIM // 128             # 16 embedding chunks
LC = LAT // 128             # 4 latent chunks
RB = N_CORES                # rank blocks of TPC tokens
SBLK = 512                  # query block (phase B)
NSB = S // SBLK             # 4 s-blocks per batch
TC_B = S // 128             # 16 key chunks per batch
F8 = mybir.dt.float8e4
F16 = mybir.dt.float16
F32 = mybir.dt.float32
SCALE = 1.0 / math.sqrt(HD)
DR = mybir.MatmulPerfMode.DoubleRow
FP8_UP = True   # fp8 DoubleRow kc/qc up-projections + fp8 c_q AllGather

_CACHE = {}

OPTS = dict()


def _build(use_cc=True, n_devices=N_CORES, reps=1, tiny_out=False,
           **opts):
    nc = bacc.Bacc("TRN2", target_bir_lowering=False, debug=False,
                   num_devices=n_devices)

    # ---- per-core external inputs (host pre-permuted into SBUF layout so
    # every input DMA is a contiguous [128, X] copy) ----
    xT = nc.dram_tensor("xT", [128, EC, TPC], F16, kind="ExternalInput")
    wdkv = nc.dram_tensor("wdkv", [128, EC, LAT], F16, kind="ExternalInput")
    wdq = nc.dram_tensor("wdq", [128, EC, LAT], F8 if FP8_UP else F16,
                         kind="ExternalInput")
    wkr = nc.dram_tensor("wkr", [128, EC, HD], F16, kind="ExternalInput")
    wqr = nc.dram_tensor("wqr", [128, EC, HD], F16, kind="ExternalInput")
    bdkv = nc.dram_tensor("bdkv", [128, LC], F32, kind="ExternalInput")
    bdq = nc.dram_tensor("bdq", [128, LC], F32, kind="ExternalInput")
    bkr = nc.dram_tensor("bkr", [128, 1], F32, kind="ExternalInput")
    bqr = nc.dram_tensor("bqr", [128, 1], F32, kind="ExternalInput")
    pet = nc.dram_tensor("pet", [HD, TPC], F32, kind="ExternalInput")
    FUP = F8 if FP8_UP else F16
    wuk = nc.dram_tensor("wuk", [128, LC, HPC * HD], FUP,
                         kind="ExternalInput")
    wuv = nc.dram_tensor("wuv", [128, LC, HPC * HD], F16,
                         kind="ExternalInput")
    wuq = nc.dram_tensor("wuq", [128, LC, HPC * HD], FUP,
                         kind="ExternalInput")
    wuk16 = nc.dram_tensor("wuk16", [128, LC, HPC * HD], F16,
                           kind="ExternalInput")
    buk = nc.dram_tensor("buk", [128, HPC], F32, kind="ExternalInput")
    buv = nc.dram_tensor("buv", [128, HPC], F32, kind="ExternalInput")
    buq = nc.dram_tensor("buq", [128, HPC], F32, kind="ExternalInput")
    wo = nc.dram_tensor("wo", [128, HPC, DIM], F16, kind="ExternalInput")

    # timing-only builds use a tiny aliased output to avoid the host
    # cost of 16MB/core output buffers per dispatch
    out_shape = [128, DIM] if tiny_out else [TOK, DIM]
    out_part = nc.dram_tensor("out_part", out_shape, F16,
                              kind="ExternalOutput")

    CKV_N = LAT * TPC                 # 262144 f16 elems per rank shard
    ROPE8 = HD * TPC                  # 65536 fp8 bytes per rope tensor
    # bounce2: c_q section (fp8 or fp16) then the two fp8 rope sections;
    # offsets in f16-element units of the bounce buffer
    CQ_SLOTS = LAT * TPC // 2 if FP8_UP else LAT * TPC
    KR_OFF = CQ_SLOTS
    QR_OFF = KR_OFF + ROPE8 // 2      # fp8 stored as half as many f16 slots
    AUX_N = QR_OFF + ROPE8 // 2

    with tile.TileContext(nc) as tc:
        def emit(rep):
                with tc.tile_pool(name=f"dram{rep}", bufs=1, space="DRAM") as dram:
                    bin1 = dram.tile([1, CKV_N], F16)
                    bout1 = dram.tile([RB, CKV_N], F16, addr_space="Shared")
                    bin2 = dram.tile([1, AUX_N], F16)
                    bout2 = dram.tile([RB, AUX_N], F16, addr_space="Shared")

                    # ============ Phase A (token shard, transposed outputs) ========
                    with tc.tile_pool(name=f"pA{rep}", bufs=1) as pA, \
                         tc.tile_pool(name=f"psA{rep}", bufs=1, space="PSUM") as psA:
                        # dual DMA queues: x streams on the SP queue while all
                        # weights stream in parallel on the ACT queue (ACT is
                        # compute-idle during phase A)
                        x_sb = pA.tile([128, EC, TPC], F16)
                        wdkv_sb = pA.tile([128, EC, LAT], F16)
                        # fine-grained leading chunks so the first matmul
                        # starts as early as possible; coarser after
                        chunks = [(0, 1), (1, 1), (2, 2), (4, 2), (6, 2),
                                  (8, 2), (10, 2), (12, 2), (14, 2)]
                        for _c, _w in chunks:
                            nc.sync.dma_start(out=x_sb[:, _c:_c + _w],
                                              in_=xT[:, _c:_c + _w, :])
                            nc.scalar.dma_start(out=wdkv_sb[:, _c:_c + _w],
                                                in_=wdkv[:, _c:_c + _w, :])
                        bdkv_sb = pA.tile([128, LC], F32)
                        nc.sync.dma_start(out=bdkv_sb, in_=bdkv[:, :])
                        if FP8_UP:
                            # fp8 copy of x for the c_q DoubleRow path;
                            # DVE is idle during phase A
                            x8_sb = pA.tile([128, EC, TPC], F8)
                            for _c in range(EC):
                                nc.vector.tensor_copy(x8_sb[:, _c],
                                                      x_sb[:, _c])
                        wdq_sb = pA.tile([128, EC, LAT],
                                         F8 if FP8_UP else F16)
                        for _c in range(0, EC, 4):
                            nc.scalar.dma_start(out=wdq_sb[:, _c:_c + 4],
                                                in_=wdq[:, _c:_c + 4, :])
                        wkr_sb = pA.tile([128, EC, HD], F16)
                        nc.scalar.dma_start(out=wkr_sb, in_=wkr[:, :, :])
                        wqr_sb = pA.tile([128, EC, HD], F16)
                        nc.scalar.dma_start(out=wqr_sb, in_=wqr[:, :, :])
                        bdq_sb = pA.tile([128, LC], F32)
                        nc.sync.dma_start(out=bdq_sb, in_=bdq[:, :])
                        bkr_sb = pA.tile([128, 1], F32)
                        nc.sync.dma_start(out=bkr_sb, in_=bkr[:, :])
                        bqr_sb = pA.tile([128, 1], F32)
                        nc.sync.dma_start(out=bqr_sb, in_=bqr[:, :])
                        pet_sb = pA.tile([128, TPC], F32)
                        nc.sync.dma_start(out=pet_sb, in_=pet[:, :])

                        # c_kvT: ec-outer so each x/wdkv chunk is consumed on arrival
                        ckvT_sb = pA.tile([128, LC, TPC], F16)
                        ps_kv = [psA.tile([128, TPC], F32, tag=f"psa{lc}",
                                          name=f"ps_kv{lc}")
                                 for lc in range(LC)]
                        for ec in range(EC):
                            for lc in range(LC):
                                nc.tensor.matmul(
                                    ps_kv[lc][:, :],
                                    wdkv_sb[:, ec, lc * 128:(lc + 1) * 128],
                                    x_sb[:, ec, :],
                                    start=(ec == 0), stop=(ec == EC - 1))
                        bin1v = bin1[0, :].rearrange("(p n f) -> p n f", p=128, f=TPC)
                        for lc in range(LC):
                            nc.scalar.add(ckvT_sb[:, lc, :], ps_kv[lc][:, :],
                                          bdkv_sb[:, lc:lc + 1])
                            nc.sync.dma_start(out=bin1v[:, lc, :],
                                              in_=ckvT_sb[:, lc, :])
                        if use_cc:
                            nc.gpsimd.collective_compute(
                                "AllGather", mybir.AluOpType.bypass,
                                replica_groups=[list(range(N_CORES))],
                                ins=[bin1.opt()], outs=[bout1.opt()])

                        # c_qT + ropes (fp8 out), ec-outer
                        cqT_sb = pA.tile([128, LC, TPC],
                                         F8 if FP8_UP else F16)
                        krT8_sb = pA.tile([128, TPC], F8)
                        qrT8_sb = pA.tile([128, TPC], F8)
                        ps_q = [psA.tile([128, TPC], F32, tag=f"psa{lc}",
                                         name=f"ps_q{lc}")
                                for lc in range(LC)]
                        ps_kr = psA.tile([128, TPC], F32, tag="psa_kr")
                        ps_qr = psA.tile([128, TPC], F32, tag="psa_qr")
                        for ec in range(EC):
                            if FP8_UP:
                                if ec % 2 == 0:
                                    ep = slice(ec, ec + 2)
                                    for lc in range(LC):
                                        nc.tensor.matmul(
                                            ps_q[lc][:, :],
                                            wdq_sb[:, ep,
                                                   lc * 128:(lc + 1) * 128],
                                            x8_sb[:, ep, :],
                                            start=(ec == 0),
                                            stop=(ec == EC - 2),
                                            perf_mode=DR)
                            else:
                                for lc in range(LC):
                                    nc.tensor.matmul(
                                        ps_q[lc][:, :],
                                        wdq_sb[:, ec, lc * 128:(lc + 1) * 128],
                                        x_sb[:, ec, :],
                                        start=(ec == 0), stop=(ec == EC - 1))
                            nc.tensor.matmul(ps_kr[:, :], wkr_sb[:, ec, :],
                                             x_sb[:, ec, :],
                                             start=(ec == 0), stop=(ec == EC - 1))
                            nc.tensor.matmul(ps_qr[:, :], wqr_sb[:, ec, :],
                                             x_sb[:, ec, :],
                                             start=(ec == 0), stop=(ec == EC - 1))
                        for lc in range(LC):
                            nc.scalar.add(cqT_sb[:, lc, :], ps_q[lc][:, :],
                                          bdq_sb[:, lc:lc + 1])
                        tmpr = pA.tile([128, TPC], F32)
                        nc.scalar.add(tmpr[:, :], ps_kr[:, :], bkr_sb[:, 0:1])
                        nc.vector.tensor_mul(krT8_sb[:, :], tmpr[:, :], pet_sb[:, :])
                        tmpr2 = pA.tile([128, TPC], F32)
                        nc.scalar.add(tmpr2[:, :], ps_qr[:, :], bqr_sb[:, 0:1])
                        nc.vector.tensor_mul(qrT8_sb[:, :], tmpr2[:, :], pet_sb[:, :])

                        cq_bounce = bin2[0, 0:KR_OFF]
                        if FP8_UP:
                            cq_bounce = cq_bounce.bitcast(F8)
                        nc.sync.dma_start(
                            out=cq_bounce.rearrange(
                                "(p n f) -> p n f", p=128, f=TPC),
                            in_=cqT_sb)
                        nc.sync.dma_start(
                            out=bin2[0, KR_OFF:QR_OFF].bitcast(F8).rearrange(
                                "(p f) -> p f", p=128), in_=krT8_sb)
                        nc.sync.dma_start(
                            out=bin2[0, QR_OFF:AUX_N].bitcast(F8).rearrange(
                                "(p f) -> p f", p=128), in_=qrT8_sb)
                        if use_cc:
                            nc.gpsimd.collective_compute(
                                "AllGather", mybir.AluOpType.bypass,
                                replica_groups=[list(range(N_CORES))],
                                ins=[bin2.opt()], outs=[bout2.opt()])

                    # ============ Phase B ==========================================
                    _pB_cm = tc.tile_pool(name=f"pB{rep}", bufs=1)
                    pB = _pB_cm.__enter__()
                    # -- AG-independent weight loads issue FIRST so the DMA queue
                    # is not blocked behind the collective-dependent transfers.
                    wuk_sb = pB.tile([128, LC, HPC * HD], FUP)
                    nc.scalar.dma_start(out=wuk_sb, in_=wuk[:, :, :])
                    wuv_sb = pB.tile([128, LC, HPC * HD], F16)
                    nc.scalar.dma_start(out=wuv_sb, in_=wuv[:, :, :])
                    wuq_sb = pB.tile([128, LC, HPC * HD], FUP)
                    nc.scalar.dma_start(out=wuq_sb, in_=wuq[:, :, :])
                    wuk16_sb = pB.tile([128, LC, HPC * HD], F16)
                    nc.scalar.dma_start(out=wuk16_sb, in_=wuk16[:, :, :])
                    wo_sb = pB.tile([128, HPC, DIM], F16)
                    nc.scalar.dma_start(out=wo_sb, in_=wo[:, :, :])
                    buk_sb = pB.tile([128, HPC], F32)
                    nc.scalar.dma_start(out=buk_sb, in_=buk[:, :])
                    buv_sb = pB.tile([128, HPC], F32)
                    nc.scalar.dma_start(out=buv_sb, in_=buv[:, :])
                    buq_sb = pB.tile([128, HPC], F32)
                    nc.scalar.dma_start(out=buq_sb, in_=buq[:, :])

                    ident = pB.tile([128, 128], F16)
                    make_identity(nc, ident)

                    # fp8 pair tiles: slots 0,1 = per-head kc/qc, slot 2 = shared rope
                    kk_sb = pB.tile([128, 3, RB, TPC], F8)
                    qq_sb = pB.tile([128, 3, RB, TPC], F8)
                    v_sb = pB.tile([128, HPC, TOK // 128, 132], F16)
                    nc.vector.memset(v_sb[:, :, :, 128:129], 1.0)

                    # -- AG#1-dependent loads (per rank so up-proj streams) --
                    ckv_sb = pB.tile([128, LC, RB, TPC], F16)
                    for r in range(RB):
                        nc.sync.dma_start(
                            out=ckv_sb[:, :, r, :],
                            in_=bout1[r, :].rearrange("(p n f) -> p n f",
                                                      p=128, f=TPC))
                    # -- AG#2-dependent loads --
                    nc.sync.dma_start(
                        out=kk_sb[:, 2, :, :],
                        in_=bout2[:, KR_OFF:QR_OFF].bitcast(F8).rearrange(
                            "r (p f) -> p r f", p=128))
                    nc.sync.dma_start(
                        out=qq_sb[:, 2, :, :],
                        in_=bout2[:, QR_OFF:AUX_N].bitcast(F8).rearrange(
                            "r (p f) -> p r f", p=128))
                    cq_sb = pB.tile([128, LC, RB, TPC], FUP)
                    cq_bout = bout2[:, 0:KR_OFF]
                    if FP8_UP:
                        cq_bout = cq_bout.bitcast(F8)
                    for r in range(RB):
                        nc.sync.dma_start(
                            out=cq_sb[:, :, r, :],
                            in_=cq_bout[r].rearrange(
                                "(p n f) -> p n f", p=128, f=TPC))
                    if FP8_UP:
                        ckv8_sb = pB.tile([128, LC, RB, TPC], F8)

                    def mm_up(ps, w_sb, act_sb, rb_, hs_):
                        """kc/qc up-projection contraction over the latent
                        chunks: fp8 DoubleRow over lc pairs, or fp16."""
                        if FP8_UP:
                            for p2 in range(LC // 2):
                                lcs = slice(2 * p2, 2 * p2 + 2)
                                nc.tensor.matmul(
                                    ps[:, :], w_sb[:, lcs, hs_],
                                    act_sb[:, lcs, rb_, :],
                                    start=(p2 == 0), stop=(p2 == LC // 2 - 1),
                                    perf_mode=DR)
                        else:
                            for lc in range(LC):
                                nc.tensor.matmul(
                                    ps[:, :], w_sb[:, lc, hs_],
                                    act_sb[:, lc, rb_, :],
                                    start=(lc == 0), stop=(lc == LC - 1))

                    # ---- up-projections (batch-0 slice first) + attention.
                    # Attention needs only batch-0 kc/v and the first query
                    # block to start, so the exp backbone starts ~25us early;
                    # the remaining up-proj pieces ride the attention
                    # prologue's PE slack. PSUM budget (8 banks): psS 2x2 +
                    # pack(ctx+transpose-scratch) 1 + ps_u 2 + ps_v 1; the
                    # out-projection borrows the ps_u rotation.
                    with tc.tile_pool(name=f"pAt{rep}", bufs=2) as pAt, \
                         tc.tile_pool(name=f"psU{rep}", bufs=1,
                                      space="PSUM") as psU, \
                         tc.tile_pool(name=f"psS{rep}", bufs=2,
                                      space="PSUM") as psS, \
                         tc.tile_pool(name=f"psC{rep}", bufs=1,
                                      space="PSUM") as psC:

                        def conv_rank(rb):
                            if not FP8_UP:
                                return
                            if rb % 2 == 0:
                                nc.scalar.copy(ckv8_sb[:, :, rb, :],
                                               ckv_sb[:, :, rb, :])
                            else:
                                nc.vector.tensor_copy(ckv8_sb[:, :, rb, :],
                                                      ckv_sb[:, :, rb, :])

                        def kc_rank(rb, late):
                            for h in range(HPC):
                                hs = slice(h * HD, (h + 1) * HD)
                                psk = psU.tile([128, TPC], F32, tag="ps_u",
                                               name="psk", bufs=2)
                                if late:
                                    # fp16 path (no ckv8 convert); bias add on
                                    # DVE to keep ACT free for the exp stream
                                    for lc in range(LC):
                                        nc.tensor.matmul(
                                            psk[:, :], wuk16_sb[:, lc, hs],
                                            ckv_sb[:, lc, rb, :],
                                            start=(lc == 0),
                                            stop=(lc == LC - 1))
                                    nc.vector.tensor_scalar_add(
                                        kk_sb[:, h, rb, :], psk[:, :],
                                        buk_sb[:, h:h + 1])
                                else:
                                    mm_up(psk, wuk_sb,
                                          ckv8_sb if FP8_UP else ckv_sb,
                                          rb, hs)
                                    nc.scalar.add(kk_sb[:, h, rb, :],
                                                  psk[:, :],
                                                  buk_sb[:, h:h + 1])

                        def v_rank(rb):
                            for t4 in range(TPC // 128):
                                psv = psU.tile([128, HPC * HD], F32,
                                               tag="ps_v", name="psv",
                                               bufs=1)
                                for lc in range(LC):
                                    nc.tensor.matmul(
                                        psv[:, :],
                                        ckv_sb[:, lc, rb,
                                               t4 * 128:(t4 + 1) * 128],
                                        wuv_sb[:, lc, :],
                                        start=(lc == 0), stop=(lc == LC - 1))
                                for h in range(HPC):
                                    nc.vector.tensor_copy(
                                        v_sb[:, h, rb * 4 + t4, 0:128],
                                        psv[:, h * HD:(h + 1) * HD])

                        def qc_rank(rq, late):
                            for h in range(HPC):
                                hs = slice(h * HD, (h + 1) * HD)
                                psq = psU.tile([128, TPC], F32, tag="ps_u",
                                               name="psq", bufs=2)
                                mm_up(psq, wuq_sb, cq_sb, rq, hs)
                                if late:
                                    nc.vector.tensor_scalar_add(
                                        qq_sb[:, h, rq, :], psq[:, :],
                                        buq_sb[:, h:h + 1])
                                else:
                                    nc.scalar.add(qq_sb[:, h, rq, :],
                                                  psq[:, :],
                                                  buq_sb[:, h:h + 1])

                        for rb in range(4):
                            conv_rank(rb)
                            kc_rank(rb, late=False)
                            v_rank(rb)
                        qc_rank(0, late=False)

                        late_work = {
                            0: [lambda: kc_rank(4, True), lambda: v_rank(4),
                                lambda: kc_rank(5, True), lambda: v_rank(5)],
                            1: [lambda: kc_rank(6, True), lambda: v_rank(6),
                                lambda: qc_rank(1, True)],
                            2: [lambda: kc_rank(7, True), lambda: v_rank(7),
                                lambda: qc_rank(2, True),
                                lambda: qc_rank(3, True)],
                            3: [lambda: qc_rank(4, True),
                                lambda: qc_rank(5, True),
                                lambda: qc_rank(6, True),
                                lambda: qc_rank(7, True)],
                        }

                        out_dram = out_part.ap().rearrange(
                            "(n p) f -> p n f", p=128)
                        n_slots = 1 if tiny_out else TOK // 128

                        def pair(h):
                            # slots {h, 2}: (kc_h, kr) / (qc_h, qr)
                            return slice(h, 3, 2 - h)

                        def issue_scores(b, sb4, h):
                            """DR-fp8 scores + exp for one block."""
                            rq = 4 * b + sb4
                            est_sb = pAt.tile([128, TC_B, SBLK], F8,
                                              tag="est", bufs=3)
                            for tp in range(TC_B // 2):
                                ps_s = psS.tile([128, 2, SBLK], F32,
                                                tag="ps_s")
                                for ti in range(2):
                                    t = 2 * tp + ti
                                    rk = 4 * b + t // 4
                                    ko = (t % 4) * 128
                                    nc.tensor.matmul(
                                        ps_s[:, ti, :],
                                        kk_sb[:, pair(h), rk, ko:ko + 128],
                                        qq_sb[:, pair(h), rq, :],
                                        start=True, stop=True, perf_mode=DR)
                                nc.scalar.activation(
                                    est_sb[:, 2 * tp:2 * (tp + 1), :],
                                    ps_s[:, :, :],
                                    mybir.ActivationFunctionType.Exp,
                                    scale=SCALE)
                            return est_sb

                        def issue_ctx_chains(b, sb4, h, est_sb):
                            """ctx matmul chains + normalize; returns the
                            normalized 128-row tiles for later transpose."""
                            muls = []
                            for sp in range(SBLK // 256):
                                ps_c = psC.tile([128, 2, 132], F32,
                                                tag="ps_c")
                                for si in range(2):
                                    ss = 2 * sp + si
                                    for t in range(TC_B):
                                        nc.tensor.matmul(
                                            ps_c[:, si, 0:129],
                                            est_sb[:, t,
                                                   ss * 128:(ss + 1) * 128],
                                            v_sb[:, h, TC_B * b + t, 0:129],
                                            start=(t == 0),
                                            stop=(t == TC_B - 1))
                                    recip = pAt.tile([128, 1], F32,
                                                     tag="recip", bufs=8)
                                    nc.vector.reciprocal(
                                        recip, ps_c[:, si, 128:129])
                                    ctxn_sb = pAt.tile([128, 128], F16,
                                                       tag="ctxn", bufs=8)
                                    nc.vector.tensor_scalar_mul(
                                        ctxn_sb[:, :], ps_c[:, si, 0:128],
                                        recip)
                                    muls.append((ss, ctxn_sb))
                            return muls

                        def issue_transposes(h, ctxT_sb, muls):
                            # deferred a full block: the DVE muls are long
                            # done, so the PE never waits here. Scratch
                            # borrows the ps_v bank (v-path is idle then).
                            sc = psU.tile([128, HPC * HD], F32, tag="ps_v",
                                          name="sc", bufs=1)
                            for i, (ss, ctxn_sb) in enumerate(muls):
                                scratch = sc[:, (i % 2) * 64:
                                             (i % 2) * 64 + 64].bitcast(F16)
                                nc.tensor.transpose(scratch, ctxn_sb[:, :],
                                                    ident[:, :])
                                nc.vector.tensor_scalar_add(
                                    ctxT_sb[:, h, ss, :], scratch,
                                    buv_sb[:, h:h + 1])

                        def issue_outproj(b, sb4, ctxT_sb):
                            rq = 4 * b + sb4
                            n0 = (rq * TPC) // 128
                            for ss in range(SBLK // 128):
                                out_sb = pAt.tile([128, DIM], F16,
                                                  tag="out", bufs=3)
                                for dt4 in range(DIM // 512):
                                    ps_o = psU.tile([128, TPC], F32,
                                                    tag="ps_u", name="ps_o",
                                                    bufs=2)
                                    for h in range(HPC):
                                        nc.tensor.matmul(
                                            ps_o[:, :],
                                            ctxT_sb[:, h, ss, :],
                                            wo_sb[:, h,
                                                  dt4 * 512:(dt4 + 1) * 512],
                                            start=(h == 0),
                                            stop=(h == HPC - 1))
                                    nc.vector.tensor_copy(
                                        out_sb[:, dt4 * 512:(dt4 + 1) * 512],
                                        ps_o[:, :])
                                nc.sync.dma_start(
                                    out=out_dram[:, (n0 + ss) % n_slots, :],
                                    in_=out_sb)

                        blocks = [(b, sb4, h) for b in range(B)
                                  for sb4 in range(NSB) for h in range(HPC)]
                        prev = None    # awaiting ctx chains
                        prev2 = None   # awaiting transposes
                        pending_out = []
                        for i, blk in enumerate(blocks):
                            b, sb4, h = blk
                            if h == 0:
                                ctxT_sb = pAt.tile(
                                    [128, HPC, SBLK // 128, 128], F16,
                                    tag="ctxT", bufs=4)
                            est = issue_scores(b, sb4, h)
                            for piece in late_work.get(i, []):
                                piece()
                            if prev is not None:
                                pb, psb4, ph, pest, pctxT = prev
                                pmuls = issue_ctx_chains(pb, psb4, ph, pest)
                                if prev2 is not None:
                                    p2h, p2ctxT, p2muls, p2b, p2sb4 = prev2
                                    issue_transposes(p2h, p2ctxT, p2muls)
                                    if p2h == 1:
                                        pending_out.append(
                                            (p2b, p2sb4, p2ctxT))
                                prev2 = (ph, pctxT, pmuls, pb, psb4)
                            if i >= 6 and pending_out:
                                issue_outproj(*pending_out.pop(0))
                            prev = (b, sb4, h, est, ctxT_sb)
                        pb, psb4, ph, pest, pctxT = prev
                        pmuls = issue_ctx_chains(pb, psb4, ph, pest)
                        p2h, p2ctxT, p2muls, p2b, p2sb4 = prev2
                        issue_transposes(p2h, p2ctxT, p2muls)
                        if p2h == 1:
                            pending_out.append((p2b, p2sb4, p2ctxT))
                        issue_transposes(ph, pctxT, pmuls)
                        pending_out.append((pb, psb4, pctxT))
                        while pending_out:
                            issue_outproj(*pending_out.pop(0))
                    _pB_cm.__exit__(None, None, None)


        for rep in range(reps):
            emit(rep)
    nc.compile()
    return nc


def _rope_pe():
    pos = np.arange(S, dtype=np.float32)[:, None]
    div = np.exp(np.arange(0, HD, 2, dtype=np.float32)
                 * (-math.log(10000.0) / HD))
    pe = np.zeros((S, HD), dtype=np.float32)
    pe[:, 0::2] = np.sin(pos * div)
    pe[:, 1::2] = np.cos(pos * div)
    return pe


def _sbl(w, f16=True):
    """[n*128, C...] -> SBUF layout [128, n, C...] (partition-major)."""
    w = np.asarray(w, np.float32)
    n = w.shape[0] // 128
    out = np.ascontiguousarray(
        w.reshape(n, 128, *w.shape[1:]).swapaxes(0, 1))
    return out.astype(np.float16) if f16 else out


def _sblb(b):
    """bias [n*128] -> [128, n] fp32."""
    b = np.asarray(b, np.float32)
    n = b.size // 128
    return np.ascontiguousarray(b.reshape(n, 128).T)


def _to8(a):
    """fp32 -> TRN e4m3 (ml_dtypes.float8_e4m3, clipped to +-240)."""
    import ml_dtypes
    return np.clip(np.asarray(a, np.float32), -240.0, 240.0).astype(
        ml_dtypes.float8_e4m3)


def _prep_in_maps(inputs):
    f16 = np.float16
    x = np.asarray(inputs["x"], np.float32).reshape(TOK, DIM)
    pe = _rope_pe()
    wdq_l = _sbl(inputs["W_DQ"], f16=False)
    shared = dict(
        wdkv=_sbl(inputs["W_DKV"]),
        wdq=_to8(wdq_l) if FP8_UP else wdq_l.astype(np.float16),
        wkr=_sbl(inputs["W_KR"]),
        wqr=_sbl(inputs["W_QR"]),
        bdkv=_sblb(inputs["b_DKV"]),
        bdq=_sblb(inputs["b_DQ"]),
        bkr=_sblb(inputs["b_KR"]),
        bqr=_sblb(inputs["b_QR"]),
    )
    in_maps = []
    for r in range(N_CORES):
        tok = slice(r * TPC, (r + 1) * TPC)
        hslice = slice(r * HPC * HD, (r + 1) * HPC * HD)
        pos0 = (r * TPC) % S
        m = dict(shared)
        # xT sbuf layout: [128, EC, TPC]; x_sb[p, n, f] = x[tok_f, n*128+p]
        m["xT"] = np.ascontiguousarray(
            x[tok].T.reshape(EC, 128, TPC).swapaxes(0, 1)).astype(f16)
        m["pet"] = np.ascontiguousarray(pe[pos0:pos0 + TPC].T)
        wuk_l = _sbl(np.asarray(inputs["W_UK"], np.float32)[:, hslice],
                     f16=False)
        wuq_l = _sbl(np.asarray(inputs["W_UQ"], np.float32)[:, hslice],
                     f16=False)
        m["wuk"] = _to8(wuk_l) if FP8_UP else wuk_l.astype(f16)
        m["wuk16"] = wuk_l.astype(f16)
        m["wuq"] = _to8(wuq_l) if FP8_UP else wuq_l.astype(f16)
        m["wuv"] = _sbl(np.asarray(inputs["W_UV"], np.float32)[:, hslice])
        m["buk"] = _sblb(np.asarray(inputs["b_UK"], np.float32)[hslice])
        m["buv"] = _sblb(np.asarray(inputs["b_UV"], np.float32)[hslice])
        m["buq"] = _sblb(np.asarray(inputs["b_UQ"], np.float32)[hslice])
        m["wo"] = _sbl(np.asarray(inputs["W_O"], np.float32)[hslice, :])
        in_maps.append(m)
    return in_maps


def _build_single(**opts):
    """Single-core, collective-free variant for cost-model timing."""
    return _build(use_cc=False, n_devices=1, **opts)


def _get_exec():
    """Build (once) a jitted shard_map executor over the 8 cores, mirroring
    concourse.bass2jax.run_bass_via_pjrt but cached so repeated kernel()
    calls do not re-trace/re-compile."""
    if "exec" in _CACHE:
        return _CACHE["exec"]
    import jax
    from jax.sharding import Mesh, PartitionSpec, NamedSharding
    from jax.experimental.shard_map import shard_map
    from concourse import bass2jax

    bass2jax.install_neuronx_cc_hook()
    if "nc" not in _CACHE:
        _CACHE["nc"] = _build()
    nc = _CACHE["nc"]

    _pname = nc.partition_id_tensor.name if nc.partition_id_tensor else None
    in_names, out_names, out_avals, zero_outs = [], [], [], []
    for alloc in nc.m.functions[0].allocations:
        if not isinstance(alloc, mybir.MemoryLocationSet):
            continue
        name = alloc.memorylocations[0].name
        if alloc.kind == "ExternalInput":
            if name != _pname:
                in_names.append(name)
        elif alloc.kind == "ExternalOutput":
            out_names.append(name)
            shape = tuple(alloc.tensor_shape)
            dtype = mybir.dt.np(alloc.dtype)
            out_avals.append(jax.core.ShapedArray(shape, dtype))
            zero_outs.append(np.zeros((N_CORES * shape[0], *shape[1:]), dtype))
    n_params = len(in_names)
    partition_name = (nc.partition_id_tensor.name
                      if nc.partition_id_tensor else None)
    all_names = in_names + out_names
    if partition_name is not None:
        all_names = all_names + [partition_name]

    def _body(*args):
        operands = list(args)
        if partition_name is not None:
            operands.append(bass2jax.partition_id_tensor())
        outs = bass2jax._bass_exec_p.bind(
            *operands,
            out_avals=tuple(out_avals),
            in_names=tuple(all_names),
            out_names=tuple(out_names),
            lowering_input_output_aliases=(),
            sim_require_finite=True,
            sim_require_nnan=True,
            nc=nc,
        )
        return tuple(outs)

    devices = jax.devices()[:N_CORES]
    mesh = Mesh(np.asarray(devices), ("core",))
    spec = PartitionSpec("core")
    in_specs = (spec,) * (n_params + len(out_names))
    out_specs = (spec,) * len(out_names)
    sharded = jax.jit(
        shard_map(_body, mesh=mesh, in_specs=in_specs, out_specs=out_specs,
                  check_rep=False),
        keep_unused=True,
    )
    sharding = NamedSharding(mesh, spec)
    zeros_dev = [jax.device_put(z, sharding) for z in zero_outs]
    _CACHE["exec"] = (sharded, in_names, out_names, out_avals, zeros_dev,
                      sharding)
    return _CACHE["exec"]


def _execute(in_maps):
    import jax
    sharded, in_names, out_names, out_avals, zeros_dev, sharding = _get_exec()
    concat_in = [
        np.concatenate([np.asarray(in_maps[c][n]) for c in range(N_CORES)],
                       axis=0)
        for n in in_names
    ]
    dev_in = [jax.device_put(a, sharding) for a in concat_in]
    out_arrs = sharded(*dev_in, *zeros_dev)
    out_arrs = [np.asarray(o) for o in out_arrs]
    return [
        {n: out_arrs[i].reshape(N_CORES, *out_avals[i].shape)[c]
         for i, n in enumerate(out_names)}
        for c in range(N_CORES)
    ]


def run(**inputs):
    in_maps = _prep_in_maps(inputs)
    results = _execute(in_maps)
    acc = np.zeros((TOK, DIM), np.float32)
    for r in range(N_CORES):
        acc += results[r]["out_part"].astype(np.float32)
    acc += np.asarray(inputs["b_O"], np.float32)
    return acc.reshape(B, S, DIM), results


def exec_only(in_maps):
    """For timing: run the prebuilt executor on preprocessed inputs."""
    return _execute(in_maps)


def timeit(inputs, n=10):
    """Time the device execution with device-resident inputs (excludes
    host prep and H2D transfer; includes PJRT/tunnel dispatch)."""
    import time
    import jax
    in_maps = _prep_in_maps(inputs)
    sharded, in_names, _, _, zeros_dev, sharding = _get_exec()
    dev_in = [
        jax.device_put(
            np.concatenate([np.asarray(in_maps[c][nm])
                            for c in range(N_CORES)], axis=0), sharding)
        for nm in in_names
    ]
    outs = sharded(*dev_in, *zeros_dev)   # warm-up
    jax.block_until_ready(outs)
    times = []
    for _ in range(n):
        t0 = time.perf_counter()
        outs = sharded(*dev_in, *zeros_dev)
        jax.block_until_ready(outs)
        times.append(time.perf_counter() - t0)
    return times


def kernel(**inputs):
    out, _ = run(**inputs)
    return out

